# revision 1
# baseline (speedup 1.0000x reference)
"""Trainium2 Bass kernel for CRFExtensionModule (conv3x3 backbone + 5 mean-field
CRF iterations with separable Gaussian blur).

Strategy (per NeuronCore, 2 images of the 16-image batch):
  - C=2 softmax collapses: with d = logit1 - logit0 the whole CRF loop is a
    single-plane recurrence  d' = du + blur(tanh(d/2))  (since
    2*blur(sigmoid(d)) - blur(ones) = blur(2*sigmoid(d)-1) = blur(tanh(d/2))).
  - conv3x3 computes the planes u1 (set0) and du = u1-u0 (set1) via banded
    matmuls, band stationary.  All 4 row-tiles are covered by ONE compound
    matmul per (c, set, kx) into a 4-bank PSUM tile -> 1 LDWEIGHTS per band
    instead of 4 (LDWEIGHTS is serialized with MATMUL on TRN2, so LDW count
    is first-order cost).  Channel-outer loop so conv starts when channel 0's
    cast-DMA lands.  Tiny K=35 "fix" matmuls handle the 2 rows per tile
    boundary, also compound.
  - blur = two *transposing* banded matmul passes on the TensorEngine.
    Output lands back in [h, w] layout - no explicit transposes anywhere.
    Loops are ordered t-outer so pass1 starts after the first tanh tile and
    pass2 starts after the first ut copy.
  - du is injected into each iteration's PSUM with one compound identity
    matmul (moving operand = du in fp16).
  - Final iteration uses 1/sqrt(2)-scaled bands (B = blur(tanh)/2) and
    recombines  out1 = B + G1,  out0 = G0 - B  where G1 = u1 + blur(ones)/2
    (computed during extraction: G1 = PSUM + (ob/2 + b1) const) and
    G0 = G1 - du.
  - Matmul operands are fp16; accumulation fp32 in PSUM.

kernel(**inputs) takes the FULL inputs and returns the FULL output.
"""

import os
import sys
from contextlib import ExitStack

sys.path.insert(0, "/opt/trn_rl_repo")

import numpy as np
import ml_dtypes

import concourse.bass as bass
import concourse.bacc as bacc
import concourse.tile as tile
import concourse.mybir as mybir
from concourse.bass_utils import run_bass_kernel_spmd

F32 = mybir.dt.float32
F32R = mybir.dt.float32r
BF16 = mybir.dt.bfloat16
FP16 = mybir.dt.float16

N_CORES = 8
IMGS_PER_CORE = 2
H = W = 512
NT = 4  # 128-row tiles per image plane
N_ITER = 5
FILT = 11


def _gauss_k():
    d = np.arange(FILT, dtype=np.float32) - np.float32((FILT - 1) / 2.0)
    k = np.exp(-(d ** 2) / np.float32(2.0)).astype(np.float32)
    return (k / k.sum()).astype(np.float32)


def _make_A(scale):
    """A[h, h'] = k[h-h'+5] for |h-h'| <= 5 (zero-padded 'SAME' 1D blur)."""
    k = (_gauss_k() * np.float32(scale)).astype(np.float32)
    A = np.zeros((H, H), np.float32)
    hp = np.arange(H)
    for j in range(FILT):
        h = hp + (j - 5)
        m = (h >= 0) & (h < H)
        A[h[m], hp[m]] = k[j]
    return A


def _win(t):
    """h' window that rows [128t, 128t+128) of A touch."""
    return max(0, 128 * t - 5), min(H, 128 * t + 133)


# ---------------------------------------------------------------------------
# kernel body (traced once; shared SPMD program for all 8 cores)
# ---------------------------------------------------------------------------


def _build(nc, tc):
    x_d = nc.dram_tensor("x", [IMGS_PER_CORE, 3, H, W], F32, kind="ExternalInput").ap()
    y_d = nc.dram_tensor("y", [IMGS_PER_CORE, 2, H, W], F32, kind="ExternalOutput").ap()
    bands_d = nc.dram_tensor("bands", [128, 18, 128], FP16, kind="ExternalInput").ap()
    wf_d = nc.dram_tensor("wf", [35, 6, 128], FP16, kind="ExternalInput").ap()
    A1_d = nc.dram_tensor("A1", [128, NT, H], FP16, kind="ExternalInput").ap()
    Ah_d = nc.dram_tensor("Ah", [128, NT, H], FP16, kind="ExternalInput").ap()
    ident_d = nc.dram_tensor("ident", [128, 128], FP16, kind="ExternalInput").ap()
    ob2b1_d = nc.dram_tensor("ob2b1", [128, NT, W], FP16, kind="ExternalInput").ap()
    biases_d = nc.dram_tensor("biases", [128, 2], F32, kind="ExternalInput").ap()

    ALU = mybir.AluOpType
    AF = mybir.ActivationFunctionType

    with ExitStack() as ctx:
        spool = ctx.enter_context(tc.tile_pool(name="sbuf", bufs=2))
        cpool = spool
        xpool = spool
        ppool = ctx.enter_context(
            tc.tile_pool(name="psum", bufs=8, space=bass.MemorySpace.PSUM))

        def psum():
            return ppool.tile([128, 512], F32, tag="ps", name="ps")

        # --- tiny consts first (conv needs them immediately) ---
        biases = cpool.tile([128, 2], F32, tag="biases", bufs=1)
        nc.scalar.dma_start(biases[:], biases_d)
        warm = cpool.tile([128, 2, 2], F32, tag="warm", bufs=1)
        bands = cpool.tile([128, 18, 128], FP16, tag="bands", bufs=1)
        nc.scalar.dma_start(bands[:], bands_d)
        wf = cpool.tile([35, 6, 128], FP16, tag="wf", bufs=1)
        nc.scalar.dma_start(wf[:], wf_d)
        # --- big consts (needed only from iteration 0); their DMAs are
        #     issued after image 0's first-tile loads ---
        A1 = cpool.tile([128, NT, H], FP16, tag="A1", bufs=1)
        ident = cpool.tile([128, 128], FP16, tag="ident", bufs=1)
        ob2b1 = cpool.tile([128, NT, W], FP16, tag="ob2b1", bufs=1)
        Ah = cpool.tile([128, NT, H], FP16, tag="Ah", bufs=1)

        for im in range(IMGS_PER_CORE):
            # ---- x: per-channel SWDGE cast-DMAs for image 0 (conv starts as
            #      soon as channel 0 lands); one descriptor for image 1 ----
            # ---- boundary rows first on the SWDGE ring (tiny; the per-bank
            #   fix matmuls need them early):
            #   xbt[3r+c (r=0 at parts 0-2, r=1 at 32-34), b, :]
            #   r=0 -> x row 128b-1 (b>0), r=1 -> x row 128b+128 (b<3)
            xbt = xpool.tile([35, NT, W], FP16, tag=f"xb{im}", name=f"xb{im}", bufs=1)
            # full memset: partitions 3-31 are multiplied by zero weights in
            # the fix matmuls, but 0 * Inf/NaN garbage = NaN
            nc.vector.memset(xbt[:], 0.0)
            nc.gpsimd.dma_start(
                xbt[0:3, 1:NT, :],
                x_d[im, :, 127:H - 128:128, :])
            nc.gpsimd.dma_start(
                xbt[32:35, 0:NT - 1, :],
                x_d[im, :, 128::128, :])

            xt = xpool.tile([128, 3, NT, W], FP16, tag=f"xt{im}", name=f"xt{im}", bufs=1)
            if im == 0:
                # per-(c,b) SWDGE cast-DMAs, tile-major: conv tile b can start
                # as soon as its 3 chunks land.  Each dst is
                # partition-contiguous (2-level dst APs corrupt on SWDGE).
                for b in range(NT):
                    for c in range(3):
                        nc.gpsimd.dma_start(
                            xt[:, c, b, :],
                            x_d[im, c, 128 * b:128 * b + 128, :])
                nc.sync.dma_start(A1[:], A1_d)
                nc.sync.dma_start(ident[:], ident_d)
                nc.sync.dma_start(Ah[:], Ah_d)
                nc.sync.dma_start(ob2b1[:], ob2b1_d)
            else:
                nc.gpsimd.dma_start(
                    xt[:], x_d[im].rearrange("c (b p) w -> p c b w", p=128))

            # ---- conv: u1 (set0) / du (set1) into two 4-bank PSUM tiles ----
            Pb = [[None, None] for _ in range(NT)]
            # bank-burst order (all 12 MMs of one PSUM bank consecutively) --
            # cycling banks per-MM oscillates the PE clock gate (K18 mode)
            for b in range(NT):
                for set_i in range(2):
                    Pb[b][set_i] = psum()
                    n_mm = 0
                    for c in range(3):
                        for kx in (1, 0, 2):
                            # kx=0 reads x[.., j-1]: src [0,511) -> out [1,512)
                            # kx=2 reads x[.., j+1]: src [1,512) -> out [0,511)
                            sl, ol = (0, 1) if kx == 0 else (1, 0) if kx == 2 else (0, 0)
                            n = W - (1 if kx != 1 else 0)
                            nc.tensor.matmul(
                                Pb[b][set_i][:, ol:ol + n],
                                bands[:, set_i * 9 + c * 3 + kx, :],
                                xt[:, c, b, sl:sl + n],
                                start=(n_mm == 0), stop=False,
                                skip_group_check=True)
                            n_mm += 1
                    for kx in (1, 0, 2):
                        sl, ol = (0, 1) if kx == 0 else (1, 0) if kx == 2 else (0, 0)
                        n = W - (1 if kx != 1 else 0)
                        nc.tensor.matmul(
                            Pb[b][set_i][:, ol:ol + n],
                            wf[:, set_i * 3 + kx, :],
                            xbt[:, b, sl:sl + n],
                            start=False, stop=(kx == 2),
                            skip_group_check=True)

            # ---- extraction: G1 = P0 + (ob/2 + b1);  du = P1 + db (fp16) ----
            G1 = spool.tile([128, NT, W], F32, tag="G1", name=f"G1_{im}")
            du4 = spool.tile([128, NT, W], FP16, tag="du4", name=f"du4_{im}")
            for b in range(NT):
                nc.vector.tensor_scalar(
                    du4[:, b, :], Pb[b][1][:], biases[:, 0:1], None, ALU.add)
            for b in range(NT):
                nc.vector.tensor_tensor(
                    G1[:, b, :], Pb[b][0][:], ob2b1[:, b, :], ALU.add)
            # G0 = G1 - du (off critical path, GpSimd)
            G0 = spool.tile([128, NT, W], F32, tag="G0", name=f"G0_{im}")
            nc.gpsimd.tensor_sub(G0[:], G1[:], du4[:])

            # ---- CRF iterations ----
            prev_dp = None
            for it in range(N_ITER):
                s_sb = spool.tile([128, NT, W], FP16, tag="s4", name=f"s4_{it}")
                for t in range(NT):
                    if it == 0:
                        # tanh(du/2) from SBUF (LUT activations reading PSUM
                        # concurrently with DVE extraction of the same banks
                        # run ~7x slow)
                        nc.scalar.activation(
                            s_sb[:, t, :], du4[:, t, :], AF.Tanh,
                            bias=0.0, scale=0.5)
                    else:
                        nc.scalar.activation(
                            s_sb[:, t, :], prev_dp[t][:], AF.Tanh,
                            bias=0.0, scale=0.5)

                A_iter = A1 if it < N_ITER - 1 else Ah
                # pass 1: UT[w, h'] chunks (transposing banded blur along h).
                # t-outer so each tanh tile feeds 4 matmuls as soon as it lands.
                ut = spool.tile([128, NT, H], FP16, tag="ut", name=f"ut_{it}")
                for s in range(NT):
                    UTP = psum()
                    for t in range(NT):
                        lo, hi = _win(t)
                        nc.tensor.matmul(
                            UTP[:, lo:hi],
                            s_sb[:, t, 128 * s:128 * s + 128],
                            A_iter[:, t, lo:hi],
                            start=(t == 0), stop=(t == NT - 1),
                            skip_group_check=True)
                    if s % 2 == 0:
                        nc.vector.tensor_copy(ut[:, s, :], UTP[:])
                    else:
                        nc.scalar.copy(ut[:, s, :], UTP[:])

                # pass 2: V[h, w'] chunks back in row layout; s4-outer so the
                # first ut copy unblocks 4 matmuls.
                last = N_ITER - 1
                if it == last - 1:
                    # the HWDGE rings go idle after image 0's outputs and take
                    # ~10us to restart; keep them warm just before the finals
                    nc.sync.dma_start(warm[:, 0, :], biases_d)
                    nc.scalar.dma_start(warm[:, 1, :], biases_d)
                DP = []
                for tp in range(NT):
                    DPt = psum()
                    DP.append(DPt)
                    for s4 in range(NT):
                        lo, hi = _win(s4)
                        nc.tensor.matmul(
                            DPt[:, lo:hi],
                            ut[:, s4, 128 * tp:128 * tp + 128],
                            A_iter[:, s4, lo:hi],
                            start=(s4 == 0),
                            stop=(s4 == NT - 1 and it == last),
                            skip_group_check=True)
                    if it < last:
                        # d' = blur(tanh(d/2)) + du : identity inject, same bank
                        nc.tensor.matmul(
                            DPt[:], ident[:], du4[:, tp, :],
                            start=False, stop=True, skip_group_check=True)
                if it < last:
                    prev_dp = DP
                else:
                    # final: B = blur(tanh/2)/2; out1 = B + G1; out0 = G0 - B
                    for tp in range(NT):
                        o1 = spool.tile([128, W], F32, tag=f"o1_{tp}", name=f"o1_{tp}")
                        o0 = spool.tile([128, W], F32, tag=f"o0_{tp}", name=f"o0_{tp}")
                        nc.vector.tensor_add(o1[:], DP[tp][:], G1[:, tp, :])
                        nc.vector.scalar_tensor_tensor(
                            o0[:], DP[tp][:], -1.0, G0[:, tp, :],
                            ALU.mult, ALU.add)
                        ring = nc.sync if tp % 2 == 0 else nc.scalar
                        ring.dma_start(y_d[im, 1, 128 * tp:128 * tp + 128, :], o1[:])
                        ring.dma_start(y_d[im, 0, 128 * tp:128 * tp + 128, :], o0[:])


_CACHE = {}


def _get_compiled():
    if "nc" in _CACHE:
        return _CACHE["nc"]
    nc = bacc.Bacc(
        "TRN2",
        target_bir_lowering=False,
        debug=False,
        enable_asserts=False,
        num_devices=N_CORES,
    )
    with tile.TileContext(nc) as tc:
        _build(nc, tc)
    nc.compile()
    _CACHE["nc"] = nc
    return nc


def host_constants(conv_w, conv_b):
    """All weight-derived device constants, as numpy arrays."""
    w = np.asarray(conv_w, np.float32)
    b = np.asarray(conv_b, np.float32)
    sets = [w[1] + 0.0, w[1] - w[0]]  # u1-plane, du-plane (3,3,3) each

    bands = np.zeros((128, 18, 128), np.float32)
    r = np.arange(128)
    for set_i, ws in enumerate(sets):
        for c in range(3):
            for kx in range(3):
                Band = np.zeros((128, 128), np.float32)
                for ky in range(3):
                    m = r - (ky - 1)
                    ok = (m >= 0) & (m < 128)
                    Band[r[ok], m[ok]] = ws[c, ky, kx]
                bands[:, set_i * 9 + c * 3 + kx, :] = Band

    wf = np.zeros((35, 6, 128), np.float32)
    for set_i, ws in enumerate(sets):
        for kx in range(3):
            WF = np.zeros((35, 128), np.float32)
            for c in range(3):
                WF[0 + c, 0] = ws[c, 0, kx]      # r=0 rows: x row 128b-1, ky=0
                WF[32 + c, 127] = ws[c, 2, kx]   # r=1 rows: x row 128b+128, ky=2
            wf[:, set_i * 3 + kx, :] = WF

    def tile4(A):
        return np.ascontiguousarray(A.reshape(NT, 128, H).transpose(1, 0, 2))

    A1 = tile4(_make_A(1.0))
    Ah = tile4(_make_A(1.0 / np.sqrt(np.float32(2.0))))

    k = _gauss_k()
    v = np.convolve(np.ones(H, np.float32), k, mode="same").astype(np.float32)
    ob_full = np.outer(v, v).astype(np.float32)  # blur(ones), rank-1
    ob2b1_full = 0.5 * ob_full + np.float32(b[1])
    ob2b1 = np.ascontiguousarray(ob2b1_full.reshape(NT, 128, W).transpose(1, 0, 2))

    db = np.float32(b[1] - b[0])
    return {
        "bands": bands.astype(np.float16),
        "wf": wf.astype(np.float16),
        "A1": A1.astype(np.float16),
        "Ah": Ah.astype(np.float16),
        "ident": np.eye(128, dtype=np.float16),
        "ob2b1": ob2b1.astype(np.float16),
        "biases": np.tile(np.array([[db, db / 2.0]], np.float32), (128, 1)),
    }


def _install_ntff_hook_shim():
    """This container's antenv lacks axon_hooks; recreate the NTFF profile
    hook via ctypes into libaxon_pjrt.so (same ABI trn_boot.py uses).
    Only invoked for traced (profiling) runs."""
    import types
    import ctypes
    import contextlib

    try:
        from antenv.axon_hooks import get_axon_ntff_profile_hook  # noqa: F401
        return
    except ImportError:
        pass

    hook = None
    so_path = "/opt/axon/libaxon_pjrt.so"
    if os.path.exists(so_path):
        lib = ctypes.CDLL(so_path)
        if hasattr(lib, "axon_start_nrt_profile"):
            lib.axon_start_nrt_profile.argtypes = [
                ctypes.POINTER(ctypes.c_int64), ctypes.c_size_t,
            ]
            lib.axon_start_nrt_profile.restype = ctypes.c_int64
            lib.axon_stop_nrt_profile.argtypes = [ctypes.c_char_p]
            lib.axon_stop_nrt_profile.restype = ctypes.c_int64

            @contextlib.contextmanager
            def _hook(output_dir, device_ids):
                import jax

                jax.devices()
                if device_ids:
                    ids = (ctypes.c_int64 * len(device_ids))(*device_ids)
                    rc = lib.axon_start_nrt_profile(ids, len(device_ids))
                else:
                    rc = lib.axon_start_nrt_profile(None, 0)
                if rc != 0:
                    raise RuntimeError(f"axon_start_nrt_profile rc={rc}")
                try:
                    yield
                finally:
                    n = lib.axon_stop_nrt_profile(str(output_dir).encode())
                    print(f"profile: {n} file(s) written to {output_dir}", file=sys.stderr)

            hook = _hook

    import antenv

    mod = types.ModuleType("antenv.axon_hooks")
    mod.get_axon_ntff_profile_hook = lambda: hook
    mod.set_axon_ntff_profile_hook = lambda h: None
    sys.modules["antenv.axon_hooks"] = mod
    antenv.axon_hooks = mod


def kernel(x, conv_w, conv_b, _trace=False, _return_results=False):
    if _trace:
        _install_ntff_hook_shim()
    x = np.ascontiguousarray(np.asarray(x, np.float32))
    consts = host_constants(conv_w, conv_b)

    nc = _get_compiled()
    in_maps = []
    for core in range(N_CORES):
        m = {"x": np.ascontiguousarray(x[IMGS_PER_CORE * core:IMGS_PER_CORE * (core + 1)])}
        m.update(consts)
        in_maps.append(m)

    res = run_bass_kernel_spmd(nc, in_maps, core_ids=list(range(N_CORES)), trace=_trace)
    out = np.concatenate([res.results[c]["y"] for c in range(N_CORES)], axis=0).astype(np.float32)
    if _return_results:
        return out, res
    return out


if __name__ == "__main__":
    rng = np.random.default_rng(0)
    x = rng.standard_normal((16, 3, H, W), dtype=np.float32)
    w = (rng.standard_normal((2, 3, 3, 3)) * 0.1).astype(np.float32)
    b = np.zeros(2, np.float32)
    y = kernel(x=x, conv_w=w, conv_b=b)
    print("out", y.shape, y.dtype)



# revision 5
# speedup vs baseline: 1.1382x; 1.1382x over previous
"""Trainium2 Bass kernel for CRFExtensionModule (conv3x3 backbone + 5 mean-field
CRF iterations with separable Gaussian blur).

Strategy (per NeuronCore, 2 images of the 16-image batch):
  - C=2 softmax collapses: with d = logit1 - logit0 the whole CRF loop is a
    single-plane recurrence  d' = du + blur(tanh(d/2)).
  - conv3x3 computes the planes u1 (set0) and du = u1-u0 (set1) via banded
    matmuls (ky folded into a banded stationary, one 512-col stream per
    (c, kx, set, bank)).  Tiny K=35 fix matmuls patch the 2 boundary rows
    per bank.  Set1 (du) runs first so the CRF can start at ~50% of conv.
  - ~40 tiny warmup matmuls at t=0 ramp the PE clock out of its low P-state
    while the first x chunks DMA in (PE otherwise starts at half speed for
    ~3us).
  - blur = two transposing banded matmul passes on the TensorEngine; output
    lands back in [h, w] layout with no explicit transposes.
  - The two images' CRF iterations are INTERLEAVED (A/B software pipeline):
    while the PE runs image B's passes, ScalarE computes image A's next
    tanh and the DVE drains image A's pass-1 PSUM.  PSUM budget: 2-bank
    tiles, tags ps2 (pass1/conv) x2 + dp (pass2) x2 = 8 banks.
  - Extraction is batched 2 banks per instruction (fewer DVE drain stalls).
  - Final iteration uses 1/sqrt(2)-scaled bands (B = blur(tanh)/2) and
    recombines  out1 = B + G1,  out0 = S - out1  with  S = 2*G1 - du
    (S on the otherwise-idle GpSimd engine; only out1 touches PSUM).
  - Matmul operands are fp16; accumulation fp32 in PSUM.

kernel(**inputs) takes the FULL inputs and returns the FULL output.
"""

import os
import sys
from contextlib import ExitStack

sys.path.insert(0, "/opt/trn_rl_repo")

import numpy as np
import ml_dtypes

import concourse.bass as bass
import concourse.bacc as bacc
import concourse.tile as tile
import concourse.mybir as mybir
from concourse.bass_utils import run_bass_kernel_spmd

F32 = mybir.dt.float32
BF16 = mybir.dt.bfloat16
FP16 = mybir.dt.float16

N_CORES = 8
IMGS_PER_CORE = 2
H = W = 512
NT = 4  # 128-row tiles per image plane
N_ITER = 5
FILT = 11
N_WARMUP = 40


def _gauss_k():
    d = np.arange(FILT, dtype=np.float32) - np.float32((FILT - 1) / 2.0)
    k = np.exp(-(d ** 2) / np.float32(2.0)).astype(np.float32)
    return (k / k.sum()).astype(np.float32)


def _make_A(scale):
    """A[h, h'] = k[h-h'+5] for |h-h'| <= 5 (zero-padded 'SAME' 1D blur)."""
    k = (_gauss_k() * np.float32(scale)).astype(np.float32)
    A = np.zeros((H, H), np.float32)
    hp = np.arange(H)
    for j in range(FILT):
        h = hp + (j - 5)
        m = (h >= 0) & (h < H)
        A[h[m], hp[m]] = k[j]
    return A


def _win(t):
    """h' window that rows [128t, 128t+128) of A touch."""
    return max(0, 128 * t - 5), min(H, 128 * t + 133)


# ---------------------------------------------------------------------------
# kernel body (traced once; shared SPMD program for all 8 cores)
# ---------------------------------------------------------------------------


def _build(nc, tc):
    x_d = nc.dram_tensor("x", [IMGS_PER_CORE, 3, H, W], F32, kind="ExternalInput").ap()
    y_d = nc.dram_tensor("y", [IMGS_PER_CORE, 2, H, W], F32, kind="ExternalOutput").ap()
    bands_d = nc.dram_tensor("bands", [128, 18, 128], FP16, kind="ExternalInput").ap()
    wf_d = nc.dram_tensor("wf", [35, 6, 128], FP16, kind="ExternalInput").ap()
    A1_d = nc.dram_tensor("A1", [128, NT, H], FP16, kind="ExternalInput").ap()
    Ah_d = nc.dram_tensor("Ah", [128, NT, H], FP16, kind="ExternalInput").ap()
    ident_d = nc.dram_tensor("ident", [128, 128], FP16, kind="ExternalInput").ap()
    ob2b1_d = nc.dram_tensor("ob2b1", [128, NT, W], FP16, kind="ExternalInput").ap()
    biases_d = nc.dram_tensor("biases", [128, 2], F32, kind="ExternalInput").ap()

    ALU = mybir.AluOpType
    AF = mybir.ActivationFunctionType

    with ExitStack() as ctx:
        spool = ctx.enter_context(tc.tile_pool(name="sbuf", bufs=2))
        cpool = spool
        ppool = ctx.enter_context(
            tc.tile_pool(name="psum", bufs=2, space=bass.MemorySpace.PSUM))

        def ps2():
            # 2-bank PSUM tile (conv set-halves / pass1 UT halves)
            return ppool.tile([128, 2, 512], F32, tag="ps2", name="ps2")

        def dp2():
            # 2-bank PSUM tile (pass2 halves)
            return ppool.tile([128, 2, 512], F32, tag="dp2", name="dp2")

        # --- PE warmup: ~40 tiny matmuls ramp the clock during the DMA wait
        warm = cpool.tile([128, 64], FP16, tag="warm", bufs=1)
        warmdma = cpool.tile([128, 2], F32, tag="warmdma", bufs=1)
        nc.vector.memset(warm[:], 0.0)
        wps = ps2()
        for i in range(N_WARMUP):
            nc.tensor.matmul(
                wps[0:64, 0, 0:64], warm[:, 0:64], warm[:, 0:64],
                start=True, stop=True, skip_group_check=True)

        # --- tiny consts (conv needs them immediately; HWDGE rings) ---
        biases = cpool.tile([128, 2], F32, tag="biases", bufs=1)
        nc.scalar.dma_start(biases[:], biases_d)
        bands = cpool.tile([128, 18, 128], FP16, tag="bands", bufs=1)
        nc.sync.dma_start(bands[:], bands_d)
        wf = cpool.tile([35, 6, 128], FP16, tag="wf", bufs=1)
        nc.scalar.dma_start(wf[:], wf_d)

        # --- x loads: per-(c,b) SWDGE cast-DMAs, conv consumption order ---
        xt = [None, None]
        xbt = [None, None]
        for im in range(IMGS_PER_CORE):
            xt[im] = spool.tile([128, 3, NT, W], FP16, tag=f"xt{im}",
                                name=f"xt{im}", bufs=1)
            xbt[im] = spool.tile([35, NT, W], FP16, tag=f"xb{im}",
                                 name=f"xb{im}", bufs=1)
            # zero: partitions 3-31 are weight-zero in fix MMs, but 0*garbage=NaN
            nc.vector.memset(xbt[im][:], 0.0)
        for im in range(IMGS_PER_CORE):
            for b in range(NT):
                for c in range(3):
                    nc.gpsimd.dma_start(
                        xt[im][:, c, b, :],
                        x_d[im, c, 128 * b:128 * b + 128, :])
                if b == 0:
                    # boundary rows (needed by bank b's fix MMs):
                    #   parts 0-2: x row 128b-1 (b>0); parts 32-34: x row 128b+128
                    nc.gpsimd.dma_start(
                        xbt[im][0:3, 1:NT, :],
                        x_d[im, :, 127:H - 128:128, :])
                    nc.gpsimd.dma_start(
                        xbt[im][32:35, 0:NT - 1, :],
                        x_d[im, :, 128::128, :])

        # --- big consts (needed from iteration 0, after conv start) ---
        A1 = cpool.tile([128, NT, H], FP16, tag="A1", bufs=1)
        nc.sync.dma_start(A1[:], A1_d)
        ident = cpool.tile([128, 128], FP16, tag="ident", bufs=1)
        nc.scalar.dma_start(ident[:], ident_d)
        ob2b1 = cpool.tile([128, NT, W], FP16, tag="ob2b1", bufs=1)
        nc.sync.dma_start(ob2b1[:], ob2b1_d)
        Ah = cpool.tile([128, NT, H], FP16, tag="Ah", bufs=1)
        nc.scalar.dma_start(Ah[:], Ah_d)

        # =================================================================
        # Phase A: convs.  Per image: set1 (du-plane) then set0 (u1-plane),
        # so du4 extraction (and the CRF) can start at 50% of each conv.
        # =================================================================
        du4 = [None, None]
        G1 = [None, None]
        S = [None, None]

        def conv_set(im, set_i):
            """One output plane: 4 banks as 2x 2-bank psum tiles."""
            tiles = [ps2(), ps2()]
            for b in range(NT):
                P = tiles[b // 2]
                n_mm = 0
                for c in range(3):
                    for kx in (1, 0, 2):
                        # kx=0 reads x[.., j-1]: src [0,511) -> out [1,512)
                        # kx=2 reads x[.., j+1]: src [1,512) -> out [0,511)
                        sl, ol = (0, 1) if kx == 0 else (1, 0) if kx == 2 else (0, 0)
                        n = W - (1 if kx != 1 else 0)
                        nc.tensor.matmul(
                            P[:, b % 2, ol:ol + n],
                            bands[:, set_i * 9 + c * 3 + kx, :],
                            xt[im][:, c, b, sl:sl + n],
                            start=(n_mm == 0), stop=False,
                            skip_group_check=True)
                        n_mm += 1
                for kx in (1, 0, 2):
                    sl, ol = (0, 1) if kx == 0 else (1, 0) if kx == 2 else (0, 0)
                    n = W - (1 if kx != 1 else 0)
                    nc.tensor.matmul(
                        P[:, b % 2, ol:ol + n],
                        wf[:, set_i * 3 + kx, :],
                        xbt[im][:, b, sl:sl + n],
                        start=False, stop=(kx == 2),
                        skip_group_check=True)
            return tiles

        for im in range(IMGS_PER_CORE):
            # set1: du = P1 + db  (fp16)
            P1 = conv_set(im, 1)
            du4[im] = spool.tile([128, NT, W], FP16, tag=f"du4_{im}",
                                 name=f"du4_{im}", bufs=1)
            for h in range(2):
                nc.vector.tensor_scalar(
                    du4[im][:, 2 * h:2 * h + 2, :], P1[h][:],
                    biases[:, 0:1], None, ALU.add)
            # set0: G1 = P0 + (ob/2 + b1)  (fp16)
            P0 = conv_set(im, 0)
            G1[im] = spool.tile([128, NT, W], FP16, tag=f"G1_{im}",
                                name=f"G1_{im}", bufs=1)
            for h in range(2):
                nc.vector.tensor_tensor(
                    G1[im][:, 2 * h:2 * h + 2, :], P0[h][:],
                    ob2b1[:, 2 * h:2 * h + 2, :], ALU.add)
            # S = 2*G1 - du  (for out0 = S - out1; GpSimd, off critical path)
            G0 = spool.tile([128, NT, W], FP16, tag=f"G0_{im}", name=f"G0_{im}",
                            bufs=1)
            nc.gpsimd.tensor_sub(G0[:], G1[im][:], du4[im][:])
            S[im] = spool.tile([128, NT, W], FP16, tag=f"S_{im}",
                               name=f"S_{im}", bufs=1)
            nc.gpsimd.tensor_add(S[im][:], G0[:], G1[im][:])

        # =================================================================
        # Phase B: CRF iterations, images interleaved (A/B pipeline).
        # Each (im, it) section: pass1 -> extract ut -> pass2(+inject) ->
        # tanh for the next iteration (so the other image's PE work overlaps
        # this image's ScalarE tanh).
        # =================================================================
        s4 = [None, None]   # tanh(d/2) of the current iteration, per image
        DP = [None, None]   # pass2 output psum pairs, per image
        o1 = [None, None]

        # iteration-0 tanh from du4 (SBUF)
        for im in range(IMGS_PER_CORE):
            s4[im] = spool.tile([128, NT, W], FP16, tag="s4", name=f"s4_{im}0")
            for h in range(2):
                nc.scalar.activation(
                    s4[im][:, 2 * h:2 * h + 2, :], du4[im][:, 2 * h:2 * h + 2, :],
                    AF.Tanh, bias=0.0, scale=0.5)

        last = N_ITER - 1
        for it in range(N_ITER):
            for im in range(IMGS_PER_CORE):
                A_iter = A1 if it < last else Ah
                # --- pass 1: UT[w, h'] = sum_t s[:,t,:].T A[t]  (transposing)
                UTP = [ps2(), ps2()]
                for s in range(NT):
                    for t in range(NT):
                        lo, hi = _win(t)
                        nc.tensor.matmul(
                            UTP[s // 2][:, s % 2, lo:hi],
                            s4[im][:, t, 128 * s:128 * s + 128],
                            A_iter[:, t, lo:hi],
                            start=(t == 0), stop=(t == NT - 1),
                            skip_group_check=True)
                # --- extract ut (pass2 stationary must live in SBUF)
                ut = spool.tile([128, NT, H], FP16, tag="ut", name=f"ut_{im}{it}")
                nc.vector.tensor_copy(ut[:, 0:2, :], UTP[0][:])
                nc.scalar.copy(ut[:, 2:4, :], UTP[1][:])
                # --- pass 2 (+ du inject), back to [h, w] layout
                DPn = [dp2(), dp2()]
                for tp in range(NT):
                    for s4i in range(NT):
                        lo, hi = _win(s4i)
                        nc.tensor.matmul(
                            DPn[tp // 2][:, tp % 2, lo:hi],
                            ut[:, s4i, 128 * tp:128 * tp + 128],
                            A_iter[:, s4i, lo:hi],
                            start=(s4i == 0),
                            stop=(s4i == NT - 1 and it == last),
                            skip_group_check=True)
                    if it < last:
                        nc.tensor.matmul(
                            DPn[tp // 2][:, tp % 2, :], ident[:],
                            du4[im][:, tp, :],
                            start=False, stop=True, skip_group_check=True)
                DP[im] = DPn

                if it == last - 1 and im == 0:
                    # HWDGE rings idle since input loads; wake them before
                    # the finals (~10us restart penalty otherwise)
                    nc.sync.dma_start(warmdma[:, 0:1], biases_d[:, 0:1])
                    nc.scalar.dma_start(warmdma[:, 1:2], biases_d[:, 0:1])

                if it < last:
                    # tanh for the NEXT iteration (same section, so the other
                    # image's matmuls overlap this ScalarE work)
                    s4[im] = spool.tile([128, NT, W], FP16, tag="s4",
                                        name=f"s4_{im}{it + 1}")
                    for h in range(2):
                        nc.scalar.activation(
                            s4[im][:, 2 * h:2 * h + 2, :], DPn[h][:],
                            AF.Tanh, bias=0.0, scale=0.5)
                else:
                    # finals: B = blur(tanh/2)/2; out1 = B + G1; out0 = S - out1
                    o1[im] = spool.tile([128, NT, W], F32, tag="o1",
                                        name=f"o1_{im}")
                    o0 = spool.tile([128, NT, W], F32, tag="o0", name=f"o0_{im}")
                    for h in range(2):
                        nc.vector.tensor_tensor(
                            o1[im][:, 2 * h:2 * h + 2, :], DPn[h][:],
                            G1[im][:, 2 * h:2 * h + 2, :], ALU.add)
                        nc.gpsimd.tensor_tensor(
                            o0[:, 2 * h:2 * h + 2, :], S[im][:, 2 * h:2 * h + 2, :],
                            o1[im][:, 2 * h:2 * h + 2, :], ALU.subtract)
                        ring = nc.sync if h == 0 else nc.scalar
                        ring.dma_start(
                            y_d[im, 1].rearrange("(b p) w -> p b w", p=128)[:, 2 * h:2 * h + 2, :],
                            o1[im][:, 2 * h:2 * h + 2, :])
                        ring2 = nc.scalar if h == 0 else nc.sync
                        ring2.dma_start(
                            y_d[im, 0].rearrange("(b p) w -> p b w", p=128)[:, 2 * h:2 * h + 2, :],
                            o0[:, 2 * h:2 * h + 2, :])


_CACHE = {}


def _get_compiled():
    if "nc" in _CACHE:
        return _CACHE["nc"]
    nc = bacc.Bacc(
        "TRN2",
        target_bir_lowering=False,
        debug=False,
        enable_asserts=False,
        num_devices=N_CORES,
    )
    with tile.TileContext(nc) as tc:
        _build(nc, tc)
    nc.compile()
    _CACHE["nc"] = nc
    return nc


def host_constants(conv_w, conv_b):
    """All weight-derived device constants, as numpy arrays."""
    w = np.asarray(conv_w, np.float32)
    b = np.asarray(conv_b, np.float32)
    sets = [w[1] + 0.0, w[1] - w[0]]  # u1-plane, du-plane (3,3,3) each

    bands = np.zeros((128, 18, 128), np.float32)
    r = np.arange(128)
    for set_i, ws in enumerate(sets):
        for c in range(3):
            for kx in range(3):
                Band = np.zeros((128, 128), np.float32)
                for ky in range(3):
                    m = r - (ky - 1)
                    ok = (m >= 0) & (m < 128)
                    Band[r[ok], m[ok]] = ws[c, ky, kx]
                bands[:, set_i * 9 + c * 3 + kx, :] = Band

    wf = np.zeros((35, 6, 128), np.float32)
    for set_i, ws in enumerate(sets):
        for kx in range(3):
            WF = np.zeros((35, 128), np.float32)
            for c in range(3):
                WF[0 + c, 0] = ws[c, 0, kx]      # r=0 rows: x row 128b-1, ky=0
                WF[32 + c, 127] = ws[c, 2, kx]   # r=1 rows: x row 128b+128, ky=2
            wf[:, set_i * 3 + kx, :] = WF

    def tile4(A):
        return np.ascontiguousarray(A.reshape(NT, 128, H).transpose(1, 0, 2))

    A1 = tile4(_make_A(1.0))
    Ah = tile4(_make_A(1.0 / np.sqrt(np.float32(2.0))))

    k = _gauss_k()
    v = np.convolve(np.ones(H, np.float32), k, mode="same").astype(np.float32)
    ob_full = np.outer(v, v).astype(np.float32)  # blur(ones), rank-1
    ob2b1_full = 0.5 * ob_full + np.float32(b[1])
    ob2b1 = np.ascontiguousarray(ob2b1_full.reshape(NT, 128, W).transpose(1, 0, 2))

    db = np.float32(b[1] - b[0])
    return {
        "bands": bands.astype(np.float16),
        "wf": wf.astype(np.float16),
        "A1": A1.astype(np.float16),
        "Ah": Ah.astype(np.float16),
        "ident": np.eye(128, dtype=np.float16),
        "ob2b1": ob2b1.astype(np.float16),
        "biases": np.tile(np.array([[db, db / 2.0]], np.float32), (128, 1)),
    }


def _install_ntff_hook_shim():
    """This container's antenv lacks axon_hooks; recreate the NTFF profile
    hook via ctypes into libaxon_pjrt.so (same ABI trn_boot.py uses).
    Only invoked for traced (profiling) runs."""
    import types
    import ctypes
    import contextlib

    try:
        from antenv.axon_hooks import get_axon_ntff_profile_hook  # noqa: F401
        return
    except ImportError:
        pass

    hook = None
    so_path = "/opt/axon/libaxon_pjrt.so"
    if os.path.exists(so_path):
        lib = ctypes.CDLL(so_path)
        if hasattr(lib, "axon_start_nrt_profile"):
            lib.axon_start_nrt_profile.argtypes = [
                ctypes.POINTER(ctypes.c_int64), ctypes.c_size_t,
            ]
            lib.axon_start_nrt_profile.restype = ctypes.c_int64
            lib.axon_stop_nrt_profile.argtypes = [ctypes.c_char_p]
            lib.axon_stop_nrt_profile.restype = ctypes.c_int64

            @contextlib.contextmanager
            def _hook(output_dir, device_ids):
                import jax

                jax.devices()
                if device_ids:
                    ids = (ctypes.c_int64 * len(device_ids))(*device_ids)
                    rc = lib.axon_start_nrt_profile(ids, len(device_ids))
                else:
                    rc = lib.axon_start_nrt_profile(None, 0)
                if rc != 0:
                    raise RuntimeError(f"axon_start_nrt_profile rc={rc}")
                try:
                    yield
                finally:
                    n = lib.axon_stop_nrt_profile(str(output_dir).encode())
                    print(f"profile: {n} file(s) written to {output_dir}", file=sys.stderr)

            hook = _hook

    import antenv

    mod = types.ModuleType("antenv.axon_hooks")
    mod.get_axon_ntff_profile_hook = lambda: hook
    mod.set_axon_ntff_profile_hook = lambda h: None
    sys.modules["antenv.axon_hooks"] = mod
    antenv.axon_hooks = mod


def kernel(x, conv_w, conv_b, _trace=False, _return_results=False):
    if _trace:
        _install_ntff_hook_shim()
    x = np.ascontiguousarray(np.asarray(x, np.float32))
    consts = host_constants(conv_w, conv_b)

    nc = _get_compiled()
    in_maps = []
    for core in range(N_CORES):
        m = {"x": np.ascontiguousarray(x[IMGS_PER_CORE * core:IMGS_PER_CORE * (core + 1)])}
        m.update(consts)
        in_maps.append(m)

    res = run_bass_kernel_spmd(nc, in_maps, core_ids=list(range(N_CORES)), trace=_trace)
    out = np.concatenate([res.results[c]["y"] for c in range(N_CORES)], axis=0).astype(np.float32)
    if _return_results:
        return out, res
    return out


if __name__ == "__main__":
    rng = np.random.default_rng(0)
    x = rng.standard_normal((16, 3, H, W), dtype=np.float32)
    w = (rng.standard_normal((2, 3, 3, 3)) * 0.1).astype(np.float32)
    b = np.zeros(2, np.float32)
    y = kernel(x=x, conv_w=w, conv_b=b)
    print("out", y.shape, y.dtype)


# revision 11
# speedup vs baseline: 1.2128x; 1.0656x over previous
"""Trainium2 Bass kernel for CRFExtensionModule (conv3x3 backbone + 5 mean-field
CRF iterations with separable Gaussian blur).

Strategy (per NeuronCore, 2 images of the 16-image batch):
  - C=2 softmax collapses: with d = logit1 - logit0 the whole CRF loop is a
    single-plane recurrence  d' = du + blur(tanh(d/2)).
  - conv3x3 computes the planes u1 (set0) and du = u1-u0 (set1) via banded
    matmuls (ky folded into a banded stationary, one 512-col stream per
    (c, kx, set, bank)).  Tiny K=35 fix matmuls patch the 2 boundary rows
    per bank.  Set1 (du) runs first so the CRF can start at ~50% of conv.
  - ~40 tiny warmup matmuls at t=0 ramp the PE clock out of its low P-state
    while the first x chunks DMA in (PE otherwise starts at half speed for
    ~3us).
  - blur = two transposing banded matmul passes on the TensorEngine; output
    lands back in [h, w] layout with no explicit transposes.
  - The two images' CRF iterations are INTERLEAVED (A/B software pipeline):
    while the PE runs image B's passes, ScalarE computes image A's next
    tanh and the DVE drains image A's pass-1 PSUM.  PSUM budget: 2-bank
    tiles, tags ps2 (pass1/conv) x2 + dp (pass2) x2 = 8 banks.
  - Extraction is batched 2 banks per instruction (fewer DVE drain stalls).
  - Final iteration uses 1/sqrt(2)-scaled bands (B = blur(tanh)/2) and
    recombines  out1 = B + G1,  out0 = S - out1  with  S = 2*G1 - du
    (S on the otherwise-idle GpSimd engine; only out1 touches PSUM).
  - Matmul operands are fp16; accumulation fp32 in PSUM.

kernel(**inputs) takes the FULL inputs and returns the FULL output.
"""

import os
import sys
from contextlib import ExitStack

sys.path.insert(0, "/opt/trn_rl_repo")

import numpy as np
import ml_dtypes

import concourse.bass as bass
import concourse.bacc as bacc
import concourse.tile as tile
import concourse.mybir as mybir
from concourse.bass_utils import run_bass_kernel_spmd

F32 = mybir.dt.float32
BF16 = mybir.dt.bfloat16
FP16 = mybir.dt.float16

N_CORES = 8
IMGS_PER_CORE = 2
H = W = 512
NT = 4  # 128-row tiles per image plane
N_ITER = 5
FILT = 11
N_WARMUP = 40


def _gauss_k():
    d = np.arange(FILT, dtype=np.float32) - np.float32((FILT - 1) / 2.0)
    k = np.exp(-(d ** 2) / np.float32(2.0)).astype(np.float32)
    return (k / k.sum()).astype(np.float32)


def _make_A(scale):
    """A[h, h'] = k[h-h'+5] for |h-h'| <= 5 (zero-padded 'SAME' 1D blur)."""
    k = (_gauss_k() * np.float32(scale)).astype(np.float32)
    A = np.zeros((H, H), np.float32)
    hp = np.arange(H)
    for j in range(FILT):
        h = hp + (j - 5)
        m = (h >= 0) & (h < H)
        A[h[m], hp[m]] = k[j]
    return A


def _win(t):
    """h' window that rows [128t, 128t+128) of A touch."""
    return max(0, 128 * t - 5), min(H, 128 * t + 133)


# ---------------------------------------------------------------------------
# kernel body (traced once; shared SPMD program for all 8 cores)
# ---------------------------------------------------------------------------


def _build(nc, tc):
    x_d = nc.dram_tensor("x", [IMGS_PER_CORE, 3, H, W], F32, kind="ExternalInput").ap()
    y_d = nc.dram_tensor("y", [IMGS_PER_CORE, 2, H, W], F32, kind="ExternalOutput").ap()
    bands_d = nc.dram_tensor("bands", [128, 18, 128], FP16, kind="ExternalInput").ap()
    wf_d = nc.dram_tensor("wf", [35, 6, 128], FP16, kind="ExternalInput").ap()
    A1_d = nc.dram_tensor("A1", [128, NT, H], FP16, kind="ExternalInput").ap()
    Ah_d = nc.dram_tensor("Ah", [128, NT, H], FP16, kind="ExternalInput").ap()
    ident_d = nc.dram_tensor("ident", [128, 128], FP16, kind="ExternalInput").ap()
    ob2b1_d = nc.dram_tensor("ob2b1", [128, NT, W], FP16, kind="ExternalInput").ap()
    biases_d = nc.dram_tensor("biases", [128, 2], F32, kind="ExternalInput").ap()

    ALU = mybir.AluOpType
    AF = mybir.ActivationFunctionType

    with ExitStack() as ctx:
        spool = ctx.enter_context(tc.tile_pool(name="sbuf", bufs=2))
        cpool = spool
        ppool = ctx.enter_context(
            tc.tile_pool(name="psum", bufs=2, space=bass.MemorySpace.PSUM))

        def ps2():
            # 2-bank PSUM tile (conv set-halves / pass1 UT halves)
            return ppool.tile([128, 2, 512], F32, tag="ps2", name="ps2")

        def dp2():
            # 2-bank PSUM tile (pass2 halves)
            return ppool.tile([128, 2, 512], F32, tag="dp2", name="dp2")

        # --- PE warmup: ~40 tiny matmuls ramp the clock during the DMA wait
        warm = cpool.tile([128, 64], FP16, tag="warm", bufs=1)
        warmdma = cpool.tile([128, 2], F32, tag="warmdma", bufs=1)
        nc.vector.memset(warm[:], 0.0)
        wps = ps2()
        for i in range(N_WARMUP):
            nc.tensor.matmul(
                wps[0:64, 0, 0:64], warm[:, 0:64], warm[:, 0:64],
                start=True, stop=True, skip_group_check=True)

        # --- tiny consts (conv needs them immediately; HWDGE rings) ---
        biases = cpool.tile([128, 2], F32, tag="biases", bufs=1)
        nc.scalar.dma_start(biases[:], biases_d)
        bands = cpool.tile([128, 18, 128], FP16, tag="bands", bufs=1)
        nc.sync.dma_start(bands[:], bands_d)
        wf = cpool.tile([35, 6, 128], FP16, tag="wf", bufs=1)
        nc.scalar.dma_start(wf[:], wf_d)

        # --- x loads: per-(c,b) SWDGE cast-DMAs, conv consumption order.
        # Boundary rows go through HWDGE in f32 (strided gathers choke the
        # SWDGE Q7 descriptor generator and starve the conv) + DVE cast.
        xt = [None, None]
        xbt = [None, None]
        xbf = [None, None]
        for im in range(IMGS_PER_CORE):
            xt[im] = spool.tile([128, 3, NT, W], FP16, tag=f"xt{im}",
                                name=f"xt{im}", bufs=1)
            xbt[im] = spool.tile([35, NT, W], FP16, tag=f"xb{im}",
                                 name=f"xb{im}", bufs=1)
            xbf[im] = spool.tile([35, NT, W], F32, tag=f"xbf{im}",
                                 name=f"xbf{im}", bufs=1)
            # zero: partitions 3-31 are weight-zero in fix MMs, but 0*garbage=NaN
            nc.vector.memset(xbt[im][:], 0.0)
            # parts 0-2: x row 128b-1 (b>0); parts 32-34: x row 128b+128
            nc.sync.dma_start(xbf[im][0:3, 1:NT, :],
                              x_d[im, :, 127:H - 128:128, :])
            nc.sync.dma_start(xbf[im][32:35, 0:NT - 1, :],
                              x_d[im, :, 128::128, :])
            nc.vector.tensor_copy(xbt[im][0:3, 1:NT, :], xbf[im][0:3, 1:NT, :])
            nc.vector.tensor_copy(xbt[im][32:35, 0:NT - 1, :],
                                  xbf[im][32:35, 0:NT - 1, :])
        for im in range(IMGS_PER_CORE):
            for b in range(NT):
                for c in range(3):
                    nc.gpsimd.dma_start(
                        xt[im][:, c, b, :],
                        x_d[im, c, 128 * b:128 * b + 128, :])

        # --- big consts (needed from iteration 0, after conv start) ---
        A1 = cpool.tile([128, NT, H], FP16, tag="A1", bufs=1)
        nc.sync.dma_start(A1[:], A1_d)
        ident = cpool.tile([128, 128], FP16, tag="ident", bufs=1)
        nc.scalar.dma_start(ident[:], ident_d)
        ob2b1 = cpool.tile([128, NT, W], FP16, tag="ob2b1", bufs=1)
        nc.sync.dma_start(ob2b1[:], ob2b1_d)
        Ah = cpool.tile([128, NT, H], FP16, tag="Ah", bufs=1)
        nc.scalar.dma_start(Ah[:], Ah_d)

        # =================================================================
        # Phase A: convs.  Per image: set1 (du-plane) then set0 (u1-plane),
        # so du4 extraction (and the CRF) can start at 50% of each conv.
        # =================================================================
        du4 = [None, None]
        G1 = [None, None]
        S = [None, None]

        def conv_set(im, set_i):
            """One output plane: 4 banks as 2x 2-bank psum tiles.  All band
            MMs first (they only need xt chunks), boundary-fix MMs last (the
            xbt gather+cast can land while the bands stream)."""
            tiles = [ps2(), ps2()]
            for b in range(NT):
                P = tiles[b // 2]
                n_mm = 0
                for c in range(3):
                    for kx in (1, 0, 2):
                        # kx=0 reads x[.., j-1]: src [0,511) -> out [1,512)
                        # kx=2 reads x[.., j+1]: src [1,512) -> out [0,511)
                        sl, ol = (0, 1) if kx == 0 else (1, 0) if kx == 2 else (0, 0)
                        n = W - (1 if kx != 1 else 0)
                        nc.tensor.matmul(
                            P[:, b % 2, ol:ol + n],
                            bands[:, set_i * 9 + c * 3 + kx, :],
                            xt[im][:, c, b, sl:sl + n],
                            start=(n_mm == 0), stop=False,
                            skip_group_check=True)
                        n_mm += 1
            for b in range(NT):
                P = tiles[b // 2]
                for kx in (1, 0, 2):
                    sl, ol = (0, 1) if kx == 0 else (1, 0) if kx == 2 else (0, 0)
                    n = W - (1 if kx != 1 else 0)
                    nc.tensor.matmul(
                        P[:, b % 2, ol:ol + n],
                        wf[:, set_i * 3 + kx, :],
                        xbt[im][:, b, sl:sl + n],
                        start=False, stop=(kx == 2),
                        skip_group_check=True)
            return tiles

        for im in range(IMGS_PER_CORE):
            # set1: du = P1 + db  (fp16)
            P1 = conv_set(im, 1)
            du4[im] = spool.tile([128, NT, W], FP16, tag=f"du4_{im}",
                                 name=f"du4_{im}", bufs=1)
            for h in range(2):
                nc.vector.tensor_scalar(
                    du4[im][:, 2 * h:2 * h + 2, :], P1[h][:],
                    biases[:, 0:1], None, ALU.add)
            # set0: G1 = P0 + (ob/2 + b1)  (fp16)
            P0 = conv_set(im, 0)
            G1[im] = spool.tile([128, NT, W], FP16, tag=f"G1_{im}",
                                name=f"G1_{im}", bufs=1)
            for h in range(2):
                nc.vector.tensor_tensor(
                    G1[im][:, 2 * h:2 * h + 2, :], P0[h][:],
                    ob2b1[:, 2 * h:2 * h + 2, :], ALU.add)
            # G0 = G1 - du  (GpSimd, off critical path; for out0 = G0 - B)
            S[im] = spool.tile([128, NT, W], FP16, tag=f"G0_{im}",
                               name=f"G0_{im}", bufs=1)
            nc.gpsimd.tensor_sub(S[im][:], G1[im][:], du4[im][:])

        # =================================================================
        # Phase B: CRF iterations, images interleaved (A/B pipeline).
        # Each (im, it) section: pass1 -> extract ut -> pass2(+inject) ->
        # tanh for the next iteration (so the other image's PE work overlaps
        # this image's ScalarE tanh).
        # =================================================================
        s4 = [None, None]   # tanh(d/2) of the current iteration, per image
        DP = [None, None]   # pass2 output psum pairs, per image
        o1 = [None, None]

        # iteration-0 tanh from du4 (SBUF)
        for im in range(IMGS_PER_CORE):
            s4[im] = spool.tile([128, NT, W], FP16, tag="s4", name=f"s4_{im}0")
            for h in range(2):
                nc.scalar.activation(
                    s4[im][:, 2 * h:2 * h + 2, :], du4[im][:, 2 * h:2 * h + 2, :],
                    AF.Tanh, bias=0.0, scale=0.5)

        last = N_ITER - 1
        for it in range(N_ITER):
            for im in range(IMGS_PER_CORE):
                A_iter = A1 if it < last else Ah
                # --- pass 1: UT[w, h'] = sum_t s[:,t,:].T A[t]  (transposing)
                UTP = [ps2(), ps2()]
                for s in range(NT):
                    for t in range(NT):
                        lo, hi = _win(t)
                        nc.tensor.matmul(
                            UTP[s // 2][:, s % 2, lo:hi],
                            s4[im][:, t, 128 * s:128 * s + 128],
                            A_iter[:, t, lo:hi],
                            start=(t == 0), stop=(t == NT - 1),
                            skip_group_check=True)
                # --- extract ut (pass2 stationary must live in SBUF)
                ut = spool.tile([128, NT, H], FP16, tag="ut", name=f"ut_{im}{it}")
                nc.vector.tensor_copy(ut[:, 0:2, :], UTP[0][:])
                nc.vector.tensor_copy(ut[:, 2:4, :], UTP[1][:])
                # --- pass 2 (+ du inject), back to [h, w] layout.  On the
                # final iteration, each 2-bank pair's finals are emitted
                # right after its chains so they overlap the next pair's MMs.
                DPn = [dp2(), dp2()]
                if it == last:
                    o1[im] = spool.tile([128, NT, W], F32, tag="o1",
                                        name=f"o1_{im}")
                    o0 = spool.tile([128, NT, W], F32, tag="o0", name=f"o0_{im}")
                for h in range(2):
                    for tp in (2 * h, 2 * h + 1):
                        for s4i in range(NT):
                            lo, hi = _win(s4i)
                            nc.tensor.matmul(
                                DPn[h][:, tp % 2, lo:hi],
                                ut[:, s4i, 128 * tp:128 * tp + 128],
                                A_iter[:, s4i, lo:hi],
                                start=(s4i == 0),
                                stop=(s4i == NT - 1 and it == last),
                                skip_group_check=True)
                        if it < last:
                            nc.tensor.matmul(
                                DPn[h][:, tp % 2, :], ident[:],
                                du4[im][:, tp, :],
                                start=False, stop=True, skip_group_check=True)
                    if it == last:
                        # out1 = B + G1;  out0 = G0 - B   (B = this pair's PSUM)
                        nc.vector.tensor_tensor(
                            o1[im][:, 2 * h:2 * h + 2, :], DPn[h][:],
                            G1[im][:, 2 * h:2 * h + 2, :], ALU.add)
                        nc.vector.scalar_tensor_tensor(
                            o0[:, 2 * h:2 * h + 2, :], DPn[h][:], -1.0,
                            S[im][:, 2 * h:2 * h + 2, :], ALU.mult, ALU.add)
                        ring = nc.sync if h == 0 else nc.scalar
                        ring.dma_start(
                            y_d[im, 1].rearrange("(b p) w -> p b w", p=128)[:, 2 * h:2 * h + 2, :],
                            o1[im][:, 2 * h:2 * h + 2, :])
                        ring2 = nc.scalar if h == 0 else nc.sync
                        ring2.dma_start(
                            y_d[im, 0].rearrange("(b p) w -> p b w", p=128)[:, 2 * h:2 * h + 2, :],
                            o0[:, 2 * h:2 * h + 2, :])
                DP[im] = DPn

                if it == last - 1 and im == 0:
                    # HWDGE rings idle since input loads; wake them before
                    # the finals (~10us restart penalty otherwise)
                    nc.sync.dma_start(warmdma[:, 0:1], biases_d[:, 0:1])
                    nc.scalar.dma_start(warmdma[:, 1:2], biases_d[:, 0:1])

                if it < last:
                    # tanh for the NEXT iteration (same section, so the other
                    # image's matmuls overlap this ScalarE work)
                    s4[im] = spool.tile([128, NT, W], FP16, tag="s4",
                                        name=f"s4_{im}{it + 1}")
                    for h in range(2):
                        nc.scalar.activation(
                            s4[im][:, 2 * h:2 * h + 2, :], DPn[h][:],
                            AF.Tanh, bias=0.0, scale=0.5)


_CACHE = {}


def _get_compiled():
    if "nc" in _CACHE:
        return _CACHE["nc"]
    nc = bacc.Bacc(
        "TRN2",
        target_bir_lowering=False,
        debug=False,
        enable_asserts=False,
        num_devices=N_CORES,
    )
    with tile.TileContext(nc) as tc:
        _build(nc, tc)
    nc.compile()
    _CACHE["nc"] = nc
    return nc


def host_constants(conv_w, conv_b):
    """All weight-derived device constants, as numpy arrays."""
    w = np.asarray(conv_w, np.float32)
    b = np.asarray(conv_b, np.float32)
    sets = [w[1] + 0.0, w[1] - w[0]]  # u1-plane, du-plane (3,3,3) each

    bands = np.zeros((128, 18, 128), np.float32)
    r = np.arange(128)
    for set_i, ws in enumerate(sets):
        for c in range(3):
            for kx in range(3):
                Band = np.zeros((128, 128), np.float32)
                for ky in range(3):
                    m = r - (ky - 1)
                    ok = (m >= 0) & (m < 128)
                    Band[r[ok], m[ok]] = ws[c, ky, kx]
                bands[:, set_i * 9 + c * 3 + kx, :] = Band

    wf = np.zeros((35, 6, 128), np.float32)
    for set_i, ws in enumerate(sets):
        for kx in range(3):
            WF = np.zeros((35, 128), np.float32)
            for c in range(3):
                WF[0 + c, 0] = ws[c, 0, kx]      # r=0 rows: x row 128b-1, ky=0
                WF[32 + c, 127] = ws[c, 2, kx]   # r=1 rows: x row 128b+128, ky=2
            wf[:, set_i * 3 + kx, :] = WF

    def tile4(A):
        return np.ascontiguousarray(A.reshape(NT, 128, H).transpose(1, 0, 2))

    A1 = tile4(_make_A(1.0))
    Ah = tile4(_make_A(1.0 / np.sqrt(np.float32(2.0))))

    k = _gauss_k()
    v = np.convolve(np.ones(H, np.float32), k, mode="same").astype(np.float32)
    ob_full = np.outer(v, v).astype(np.float32)  # blur(ones), rank-1
    ob2b1_full = 0.5 * ob_full + np.float32(b[1])
    ob2b1 = np.ascontiguousarray(ob2b1_full.reshape(NT, 128, W).transpose(1, 0, 2))

    db = np.float32(b[1] - b[0])
    return {
        "bands": bands.astype(np.float16),
        "wf": wf.astype(np.float16),
        "A1": A1.astype(np.float16),
        "Ah": Ah.astype(np.float16),
        "ident": np.eye(128, dtype=np.float16),
        "ob2b1": ob2b1.astype(np.float16),
        "biases": np.tile(np.array([[db, db / 2.0]], np.float32), (128, 1)),
    }


def _install_ntff_hook_shim():
    """This container's antenv lacks axon_hooks; recreate the NTFF profile
    hook via ctypes into libaxon_pjrt.so (same ABI trn_boot.py uses).
    Only invoked for traced (profiling) runs."""
    import types
    import ctypes
    import contextlib

    try:
        from antenv.axon_hooks import get_axon_ntff_profile_hook  # noqa: F401
        return
    except ImportError:
        pass

    hook = None
    so_path = "/opt/axon/libaxon_pjrt.so"
    if os.path.exists(so_path):
        lib = ctypes.CDLL(so_path)
        if hasattr(lib, "axon_start_nrt_profile"):
            lib.axon_start_nrt_profile.argtypes = [
                ctypes.POINTER(ctypes.c_int64), ctypes.c_size_t,
            ]
            lib.axon_start_nrt_profile.restype = ctypes.c_int64
            lib.axon_stop_nrt_profile.argtypes = [ctypes.c_char_p]
            lib.axon_stop_nrt_profile.restype = ctypes.c_int64

            @contextlib.contextmanager
            def _hook(output_dir, device_ids):
                import jax

                jax.devices()
                if device_ids:
                    ids = (ctypes.c_int64 * len(device_ids))(*device_ids)
                    rc = lib.axon_start_nrt_profile(ids, len(device_ids))
                else:
                    rc = lib.axon_start_nrt_profile(None, 0)
                if rc != 0:
                    raise RuntimeError(f"axon_start_nrt_profile rc={rc}")
                try:
                    yield
                finally:
                    n = lib.axon_stop_nrt_profile(str(output_dir).encode())
                    print(f"profile: {n} file(s) written to {output_dir}", file=sys.stderr)

            hook = _hook

    import antenv

    mod = types.ModuleType("antenv.axon_hooks")
    mod.get_axon_ntff_profile_hook = lambda: hook
    mod.set_axon_ntff_profile_hook = lambda h: None
    sys.modules["antenv.axon_hooks"] = mod
    antenv.axon_hooks = mod


def kernel(x, conv_w, conv_b, _trace=False, _return_results=False):
    if _trace:
        _install_ntff_hook_shim()
    x = np.ascontiguousarray(np.asarray(x, np.float32))
    consts = host_constants(conv_w, conv_b)

    nc = _get_compiled()
    in_maps = []
    for core in range(N_CORES):
        m = {"x": np.ascontiguousarray(x[IMGS_PER_CORE * core:IMGS_PER_CORE * (core + 1)])}
        m.update(consts)
        in_maps.append(m)

    res = run_bass_kernel_spmd(nc, in_maps, core_ids=list(range(N_CORES)), trace=_trace)
    out = np.concatenate([res.results[c]["y"] for c in range(N_CORES)], axis=0).astype(np.float32)
    if _return_results:
        return out, res
    return out


if __name__ == "__main__":
    rng = np.random.default_rng(0)
    x = rng.standard_normal((16, 3, H, W), dtype=np.float32)
    w = (rng.standard_normal((2, 3, 3, 3)) * 0.1).astype(np.float32)
    b = np.zeros(2, np.float32)
    y = kernel(x=x, conv_w=w, conv_b=b)
    print("out", y.shape, y.dtype)
